# revision 10
# baseline (speedup 1.0000x reference)
"""Trainium2 Bass kernel for nn_HarmonicOscillatorOrbitals.

out[b, i, j] = exp(-s^2/2) * H_j(s), s = omega * x[b, i, 0], j = 0..31
(physicists' Hermite polynomials), data-parallel over 8 NeuronCores on
the leading batch axis.

Per core (8192 batches = 262144 scalars as [128 partitions, E=2048]),
the G_k = env*H_k recurrence runs entirely on DVE in *scaled fp16*:
each level is stored as Gh_k = G_k / 2^{e_k} (e_k = per-level
amplitude exponents, hardcoded), which keeps values in fp16 range and
makes every multiplier a power of two:

  q_k  = (t_hat / 2^{d_k}) * Gh_{k-1}     DVE tensor_tensor fp16, 2x mode
  r_k  = -b_k * Gh_{k-2}                  ACT scale-copy (own SBUF port)
  Gh_k = q_k + r_k                        DVE tensor_tensor fp16, 2x mode

t_hat = fp16(2*omega*x); its rounding error is repaired on the final
level with a first-order t_lo correction (dG/dt = k*G_{k-1} -
(t/4)*G_k), which brings max rel err to ~1.05e-2 of the global max
(gate 2e-2) — verified bit-exact against a numpy model of DVE's
fp32-internal/round-to-nearest-fp16 behavior.

GPSIMD is intentionally idle: it shares its SBUF port pair with DVE
(exclusive per-instruction lock), so any GPSIMD tensor work would
serialize against the DVE chain at worse per-element efficiency.

Startup is pipelined in two column halves (x DMA -> t -> sq/th ->
env -> Gh_1 per half) so the chain starts ~13us in. Output: fp16
scaled levels, level-major [128, NJ, E]; 4-level 2MB DMA groups (the
last group flushes as 2+2 levels around the correction). Host
multiplies by 2^{e_k}, upcasts to f32 and permutes to (batch, i, j).
"""

from contextlib import ExitStack

import numpy as np

import concourse.bacc as bacc
import concourse.mybir as mybir
import concourse.tile as tile
from concourse.bass_utils import run_bass_kernel_spmd

F32 = mybir.dt.float32
F16 = mybir.dt.float16
AF = mybir.ActivationFunctionType
ALU = mybir.AluOpType

NJ = 32          # number of Hermite orders
N_CORES = 8
B = 65536        # full batch
BC = B // N_CORES
E = BC * NJ // 128   # 2048 free elems per partition per core

KG = 4           # k-levels per DMA flush group

# Per-level scale exponents e_k: Gh_k = G_k / 2^{e_k}.  amp_k =
# max_{|s|<=5.1} |env*H_k| computed offline in float64, e_k = ceil(log2).
EXP = [0, 1, 2, 3, 4, 6, 8, 10, 12, 14, 16, 18, 20, 23, 25, 28,
       30, 33, 35, 37, 40, 43, 46, 48, 51, 54, 57, 59, 62, 65, 68, 71]
SIG = [float(2.0**e) for e in EXP]


def _build(e=E):
    nc = bacc.Bacc("TRN2", target_bir_lowering=False, debug=False)
    x_d = nc.dram_tensor("x", [128, e], F32, kind="ExternalInput").ap()
    om_d = nc.dram_tensor("om", [128, 1], F32, kind="ExternalInput").ap()
    out_d = nc.dram_tensor("out", [128, NJ * e], F16, kind="ExternalOutput").ap()

    n_groups = NJ // KG
    h = e // 2
    halves = [(0, h), (h, e)]
    with tile.TileContext(nc) as tc, ExitStack() as ctx:
        cpool = ctx.enter_context(tc.tile_pool(name="const", bufs=1))
        xp = ctx.enter_context(tc.tile_pool(name="xp", bufs=1))
        qp = ctx.enter_context(tc.tile_pool(name="qp", bufs=3))
        rp = ctx.enter_context(tc.tile_pool(name="rp", bufs=3))
        gp = ctx.enter_context(tc.tile_pool(name="gp", bufs=3))
        zp = ctx.enter_context(tc.tile_pool(name="zp", bufs=3))

        om2 = cpool.tile([128, 1], F32)
        nc.sync.dma_start(om2[:, :], om_d[:, :])
        nc.scalar.mul(om2[:, :], om2[:, :], 2.0)  # om2 = 2*omega

        # level-group tiles: [128, KG*e] fp16, level k in slice k%KG
        groups = [None] * n_groups

        def gh(k):
            return groups[k // KG][:, (k % KG) * e : (k % KG + 1) * e]

        def ghs(k, lo, hi):
            return groups[k // KG][:, (k % KG) * e + lo : (k % KG) * e + hi]

        def open_group(k):
            q, r = divmod(k, KG)
            if r == 0:
                groups[q] = gp.tile([128, KG * e], F16, name=f"grp{q}", tag="g")

        # ---- two-half pipelined startup ----
        x_t = xp.tile([128, e], F32)
        t_t = xp.tile([128, e], F32, name="t")
        th = xp.tile([128, e], F16, name="th")
        sq = xp.tile([128, e], F32, name="sq")
        open_group(0)
        open_group(1)
        ta = {}
        ta[1] = xp.tile([128, e], F16, name="ta1")
        for lo, hi in halves:
            nc.sync.dma_start(x_t[:, lo:hi], x_d[:, lo:hi])
            nc.scalar.mul(t_t[:, lo:hi], x_t[:, lo:hi], om2[:, 0:1])
            nc.vector.tensor_mul(sq[:, lo:hi], t_t[:, lo:hi], t_t[:, lo:hi])
            nc.scalar.copy(th[:, lo:hi], t_t[:, lo:hi])  # t_hat = fp16(t)
            nc.scalar.activation(
                groups[0][:, lo:hi], sq[:, lo:hi], AF.Exp, scale=-0.125
            )  # Gh_0 = env = exp(-t^2/8)
            nc.vector.tensor_scalar_mul(ta[1][:, lo:hi], th[:, lo:hi], 0.5)
            # Gh_1 = (t_hat/2)*env  (sigma_1 = 2), fp16 TT 2x
            nc.vector.tensor_mul(
                groups[0][:, e + lo : e + hi], ta[1][:, lo:hi],
                groups[0][:, lo:hi],
            )

        def flush(k0, k1):  # DMA levels [k0, k1] (same group) to DRAM
            q = k0 // KG
            r0, r1 = k0 % KG, k1 % KG
            nc.sync.dma_start(
                out_d[:, k0 * e : (k1 + 1) * e],
                groups[q][:, r0 * e : (r1 + 1) * e],
            )

        for k in (2, 3):
            dk = EXP[k] - EXP[k - 1]
            assert dk == 1
            b = 2.0 * (k - 1) * SIG[k - 2] / SIG[k]
            for lo, hi in halves:
                qh = qp.tile([128, h], F16, name=f"qh{k}{lo}", tag="qh")
                nc.vector.tensor_mul(
                    qh[:, :], ta[1][:, lo:hi], ghs(k - 1, lo, hi)
                )
                rh = rp.tile([128, h], F16, name=f"rh{k}{lo}", tag="rh")
                nc.scalar.mul(rh[:, :], ghs(k - 2, lo, hi), -b)
                nc.vector.tensor_add(ghs(k, lo, hi), qh[:, :], rh[:, :])

        flush(0, 3)
        # late prep (DVE is saturated from here on; these interleave with
        # the chain ahead of their first use)
        for dd in (2, 3):
            ta[dd] = xp.tile([128, e], F16, name=f"ta{dd}")
            nc.vector.tensor_scalar_mul(ta[dd][:, :], th[:, :], 0.5**dd)
        # correction prep: t_lo = t - t_hat ; w1x = 1-(t_hat/4)*t_lo ; z1a
        tl = xp.tile([128, e], F16, name="tl")
        nc.vector.scalar_tensor_tensor(
            tl[:, :], th[:, :], -1.0, t_t[:, :], ALU.mult, ALU.add
        )
        w1p = xp.tile([128, e], F16, name="w1p")
        nc.vector.tensor_mul(w1p[:, :], th[:, :], tl[:, :])
        w1x = xp.tile([128, e], F16, name="w1x")  # 1 - (t_hat/4)*t_lo
        nc.scalar.activation(w1x[:, :], w1p[:, :], AF.Copy, bias=1.0, scale=-0.25)
        c1 = float((NJ - 1.0) * SIG[NJ - 2] / SIG[NJ - 1])
        z1a = xp.tile([128, e], F16, name="z1a")
        nc.scalar.mul(z1a[:, :], tl[:, :], c1)

        z1 = None
        for k in range(4, NJ):
            open_group(k)
            dk = EXP[k] - EXP[k - 1]
            b = 2.0 * (k - 1) * SIG[k - 2] / SIG[k]
            q_t = qp.tile([128, e], F16, name=f"q{k}", tag="q")
            nc.vector.tensor_mul(q_t[:, :], ta[dk][:, :], gh(k - 1))
            r_t = rp.tile([128, e], F16, name=f"r{k}", tag="r")
            nc.scalar.mul(r_t[:, :], gh(k - 2), -b)
            if k == NJ - 1:
                break
            nc.vector.tensor_add(gh(k), q_t[:, :], r_t[:, :])
            if k % KG == KG - 1 and k < NJ - 1:
                flush(k - 3, k)
            if k == NJ - 2:
                flush(k - 2, k - 1)  # 28, 29
                # z1 = (c1*t_lo)*Gh_30 — issue before the last level's ops
                z1 = zp.tile([128, e], F16, name="z1", tag="z")
                nc.vector.tensor_mul(z1[:, :], z1a[:, :], gh(k))
                flush(k, k)  # 30

        # final level with folded t_lo correction:
        #   u = q + r (uncorrected Gh_31); Gh_31 = u*(1 - (t_hat/4)*t_lo) + z1
        u_t = zp.tile([128, e], F16, name="u", tag="z")
        nc.vector.tensor_add(u_t[:, :], q_t[:, :], r_t[:, :])
        v_t = zp.tile([128, e], F16, name="v", tag="z")
        nc.vector.tensor_mul(v_t[:, :], u_t[:, :], w1x[:, :])
        nc.vector.tensor_add(gh(NJ - 1), v_t[:, :], z1[:, :])
        flush(NJ - 1, NJ - 1)

    nc.compile()
    return nc


_CACHED_NC = None


def kernel(x: np.ndarray, omega_kernel: np.ndarray, **run_kwargs) -> np.ndarray:
    global _CACHED_NC
    assert x.shape == (B, NJ, 1) and omega_kernel.shape == (1, 1), (
        x.shape,
        omega_kernel.shape,
    )
    x = np.ascontiguousarray(x, np.float32)
    om = np.ascontiguousarray(omega_kernel, np.float32)

    if _CACHED_NC is None:
        _CACHED_NC = _build()
    nc = _CACHED_NC

    in_maps = [
        {
            "x": x[c * BC : (c + 1) * BC].reshape(128, E),
            "om": np.ascontiguousarray(np.broadcast_to(om, (128, 1))),
        }
        for c in range(N_CORES)
    ]
    res = run_bass_kernel_spmd(nc, in_maps, core_ids=list(range(N_CORES)), **run_kwargs)
    sig = np.asarray(SIG, np.float32)  # [NJ]
    full = np.empty((B, NJ, NJ), np.float32)
    for c in range(N_CORES):
        arr = np.asarray(res.results[c]["out"]).reshape(128, NJ, E)
        out3 = arr.astype(np.float32) * sig[None, :, None]
        # scalar index = p*E + col ; out[b, i, j] = out3[p, j, col]
        full[c * BC : (c + 1) * BC] = (
            out3.transpose(0, 2, 1).reshape(BC, NJ, NJ)
        )
    if run_kwargs:
        return full, res
    return full


# revision 12
# speedup vs baseline: 1.1457x; 1.1457x over previous
"""Trainium2 Bass kernel for nn_HarmonicOscillatorOrbitals.

out[b, i, j] = exp(-s^2/2) * H_j(s), s = omega * x[b, i, 0], j = 0..31
(physicists' Hermite polynomials), data-parallel over 8 NeuronCores on
the leading batch axis.

Per core (8192 batches = 262144 scalars as [128 partitions, E=2048]),
the G_k = env*H_k recurrence runs entirely on DVE in *scaled fp16*:
each level is stored as Gh_k = G_k / 2^{e_k} (e_k = per-level
amplitude exponents, hardcoded), which keeps values in fp16 range and
makes every multiplier a power of two:

  q_k  = (t_hat / 2^{d_k}) * Gh_{k-1}     DVE tensor_tensor fp16, 2x mode
  r_k  = -b_k * Gh_{k-2}                  ACT scale-copy (own SBUF port)
  Gh_k = q_k + r_k                        DVE tensor_tensor fp16, 2x mode

t_hat = fp16(2*omega*x); its rounding error is repaired on the final
level with a first-order t_lo correction (dG/dt = k*G_{k-1} -
(t/4)*G_k), which brings max rel err to ~1.05e-2 of the global max
(gate 2e-2) — verified bit-exact against a numpy model of DVE's
fp32-internal/round-to-nearest-fp16 behavior.

GPSIMD is intentionally idle: it shares its SBUF port pair with DVE
(exclusive per-instruction lock), so any GPSIMD tensor work would
serialize against the DVE chain at worse per-element efficiency.

Startup is pipelined in two column halves (x DMA -> t -> sq/th ->
env -> Gh_1 per half) so the chain starts ~13us in. Output: fp16
scaled levels, level-major [128, NJ, E]; 4-level 2MB DMA groups (the
last group flushes as 2+2 levels around the correction). Host
multiplies by 2^{e_k}, upcasts to f32 and permutes to (batch, i, j).
"""

from contextlib import ExitStack

import numpy as np

import concourse.bacc as bacc
import concourse.mybir as mybir
import concourse.tile as tile
from concourse.bass_utils import run_bass_kernel_spmd

F32 = mybir.dt.float32
F16 = mybir.dt.float16
AF = mybir.ActivationFunctionType
ALU = mybir.AluOpType

NJ = 32          # number of Hermite orders
N_CORES = 8
B = 65536        # full batch
BC = B // N_CORES
E = BC * NJ // 128   # 2048 free elems per partition per core

KG = 4           # k-levels per DMA flush group

# Per-level scale exponents e_k: Gh_k = G_k / 2^{e_k}.  amp_k =
# max_{|s|<=5.1} |env*H_k| computed offline in float64, e_k = ceil(log2).
EXP = [0, 1, 2, 3, 4, 6, 8, 10, 12, 14, 16, 18, 20, 23, 25, 28,
       30, 33, 35, 37, 40, 43, 46, 48, 51, 54, 57, 59, 62, 65, 68, 71]
SIG = [float(2.0**e) for e in EXP]


def _build(e=E):
    nc = bacc.Bacc("TRN2", target_bir_lowering=False, debug=False)
    x_d = nc.dram_tensor("x", [128, e], F32, kind="ExternalInput").ap()
    om_d = nc.dram_tensor("om", [128, 1], F32, kind="ExternalInput").ap()
    out_d = nc.dram_tensor("out", [128, NJ * e], F16, kind="ExternalOutput").ap()

    n_groups = NJ // KG
    h = e // 2
    halves = [(0, h), (h, e)]
    with tile.TileContext(nc) as tc, ExitStack() as ctx:
        cpool = ctx.enter_context(tc.tile_pool(name="const", bufs=1))
        xp = ctx.enter_context(tc.tile_pool(name="xp", bufs=1))
        qp = ctx.enter_context(tc.tile_pool(name="qp", bufs=3))
        rp = ctx.enter_context(tc.tile_pool(name="rp", bufs=3))
        gp = ctx.enter_context(tc.tile_pool(name="gp", bufs=3))
        zp = ctx.enter_context(tc.tile_pool(name="zp", bufs=3))

        om2 = cpool.tile([128, 1], F32)
        nc.sync.dma_start(om2[:, :], om_d[:, :])
        nc.scalar.mul(om2[:, :], om2[:, :], 2.0)  # om2 = 2*omega

        # level-group tiles: [128, KG*e] fp16, level k in slice k%KG
        groups = [None] * n_groups

        def gh(k):
            return groups[k // KG][:, (k % KG) * e : (k % KG + 1) * e]

        def open_group(k):
            q, r = divmod(k, KG)
            if r == 0:
                groups[q] = gp.tile([128, KG * e], F16, name=f"grp{q}", tag="g")

        # ---- two-half pipelined startup ----
        x_t = xp.tile([128, e], F32)
        t_t = xp.tile([128, e], F32, name="t")
        th = xp.tile([128, e], F16, name="th")
        sq = xp.tile([128, e], F16, name="sq")
        open_group(0)
        open_group(1)
        for lo, hi in halves:
            nc.sync.dma_start(x_t[:, lo:hi], x_d[:, lo:hi])
            nc.scalar.mul(t_t[:, lo:hi], x_t[:, lo:hi], om2[:, 0:1])
            nc.scalar.copy(th[:, lo:hi], t_t[:, lo:hi])  # t_hat = fp16(t)
            nc.vector.tensor_mul(sq[:, lo:hi], th[:, lo:hi], th[:, lo:hi])
            nc.scalar.activation(
                groups[0][:, lo:hi], sq[:, lo:hi], AF.Exp, scale=-0.125
            )  # Gh_0 = env = exp(-t_hat^2/8)
            # Gh_1 = (t/2)*env  (sigma_1 = 2)
            nc.vector.scalar_tensor_tensor(
                groups[0][:, e + lo : e + hi],
                t_t[:, lo:hi],
                0.5,
                groups[0][:, lo:hi],
                ALU.mult,
                ALU.mult,
            )

        # pre-scaled t_hat variants: th/2^d (fp16 halvings, exact);
        # ta2/ta3 are filled by ACT inside the chain loop (first use k=5/k=13)
        ta = {}
        for dd in (1, 2, 3):
            ta[dd] = xp.tile([128, e], F16, name=f"ta{dd}")
        nc.vector.tensor_scalar_mul(ta[1][:, :], th[:, :], 0.5)
        # correction prep (runs in the pre-chain DVE idle window):
        # t_lo = t - t_hat ; w1 = -(t_hat/4)*t_lo ; z1a = c1*t_lo
        tl = xp.tile([128, e], F16, name="tl")
        nc.vector.scalar_tensor_tensor(
            tl[:, :], th[:, :], -1.0, t_t[:, :], ALU.mult, ALU.add
        )
        w1p = xp.tile([128, e], F16, name="w1p")
        nc.vector.tensor_mul(w1p[:, :], th[:, :], tl[:, :])
        w1x = xp.tile([128, e], F16, name="w1x")  # 1 - (t_hat/4)*t_lo
        nc.scalar.activation(w1x[:, :], w1p[:, :], AF.Copy, bias=1.0, scale=-0.25)
        c1 = float((NJ - 1.0) * SIG[NJ - 2] / SIG[NJ - 1])
        z1a = xp.tile([128, e], F16, name="z1a")
        nc.scalar.mul(z1a[:, :], tl[:, :], c1)

        def flush(k0, k1):  # DMA levels [k0, k1] (same group) to DRAM
            q = k0 // KG
            r0, r1 = k0 % KG, k1 % KG
            nc.sync.dma_start(
                out_d[:, k0 * e : (k1 + 1) * e],
                groups[q][:, r0 * e : (r1 + 1) * e],
            )

        z1 = None
        for k in range(2, NJ):
            open_group(k)
            dk = EXP[k] - EXP[k - 1]
            b = 2.0 * (k - 1) * SIG[k - 2] / SIG[k]
            q_t = qp.tile([128, e], F16, name=f"q{k}", tag="q")
            nc.vector.tensor_mul(q_t[:, :], ta[dk][:, :], gh(k - 1))
            r_t = rp.tile([128, e], F16, name=f"r{k}", tag="r")
            nc.scalar.mul(r_t[:, :], gh(k - 2), -b)
            if k == 2:
                nc.scalar.mul(ta[2][:, :], th[:, :], 0.25)
            if k == 5:
                nc.scalar.mul(ta[3][:, :], th[:, :], 0.125)
            if k == NJ - 1:
                break
            nc.vector.tensor_add(gh(k), q_t[:, :], r_t[:, :])
            if k % KG == KG - 1 and k < NJ - 1:
                flush(k - 3, k)
            if k == NJ - 2:
                flush(k - 2, k - 1)  # 28, 29
                # z1 = (c1*t_lo)*Gh_30 — issue before the last level's ops
                z1 = zp.tile([128, e], F16, name="z1", tag="z")
                nc.vector.tensor_mul(z1[:, :], z1a[:, :], gh(k))
                flush(k, k)  # 30

        # final level with folded t_lo correction:
        #   u = q + r (uncorrected Gh_31); Gh_31 = u*(1 - (t_hat/4)*t_lo) + z1
        u_t = zp.tile([128, e], F16, name="u", tag="z")
        nc.vector.tensor_add(u_t[:, :], q_t[:, :], r_t[:, :])
        v_t = zp.tile([128, e], F16, name="v", tag="z")
        nc.vector.tensor_mul(v_t[:, :], u_t[:, :], w1x[:, :])
        nc.vector.tensor_add(gh(NJ - 1), v_t[:, :], z1[:, :])
        flush(NJ - 1, NJ - 1)

    nc.compile()
    return nc


_CACHED_NC = None


def kernel(x: np.ndarray, omega_kernel: np.ndarray, **run_kwargs) -> np.ndarray:
    global _CACHED_NC
    assert x.shape == (B, NJ, 1) and omega_kernel.shape == (1, 1), (
        x.shape,
        omega_kernel.shape,
    )
    x = np.ascontiguousarray(x, np.float32)
    om = np.ascontiguousarray(omega_kernel, np.float32)

    if _CACHED_NC is None:
        _CACHED_NC = _build()
    nc = _CACHED_NC

    in_maps = [
        {
            "x": x[c * BC : (c + 1) * BC].reshape(128, E),
            "om": np.ascontiguousarray(np.broadcast_to(om, (128, 1))),
        }
        for c in range(N_CORES)
    ]
    res = run_bass_kernel_spmd(nc, in_maps, core_ids=list(range(N_CORES)), **run_kwargs)
    sig = np.asarray(SIG, np.float32)  # [NJ]
    full = np.empty((B, NJ, NJ), np.float32)
    for c in range(N_CORES):
        arr = np.asarray(res.results[c]["out"]).reshape(128, NJ, E)
        out3 = arr.astype(np.float32) * sig[None, :, None]
        # scalar index = p*E + col ; out[b, i, j] = out3[p, j, col]
        full[c * BC : (c + 1) * BC] = (
            out3.transpose(0, 2, 1).reshape(BC, NJ, NJ)
        )
    if run_kwargs:
        return full, res
    return full


# revision 14
# speedup vs baseline: 1.1781x; 1.0282x over previous
"""Trainium2 Bass kernel for nn_HarmonicOscillatorOrbitals.

out[b, i, j] = exp(-s^2/2) * H_j(s), s = omega * x[b, i, 0], j = 0..31
(physicists' Hermite polynomials), data-parallel over 8 NeuronCores on
the leading batch axis.

Per core (8192 batches = 262144 scalars as [128 partitions, E=2048]),
the G_k = env*H_k recurrence runs entirely on DVE in *scaled fp16*:
each level is stored as Gh_k = G_k / 2^{e_k} (e_k = per-level
amplitude exponents, hardcoded), which keeps values in fp16 range and
makes every multiplier a power of two:

  q_k  = (t_hat / 2^{d_k}) * Gh_{k-1}     DVE tensor_tensor fp16, 2x mode
  r_k  = -b_k * Gh_{k-2}                  ACT scale-copy (own SBUF port)
  Gh_k = q_k + r_k                        DVE tensor_tensor fp16, 2x mode

t_hat = fp16(2*omega*x); its rounding error is repaired on the final
level with a first-order t_lo correction (dG/dt = k*G_{k-1} -
(t/4)*G_k), which brings max rel err to ~1.05e-2 of the global max
(gate 2e-2) — verified bit-exact against a numpy model of DVE's
fp32-internal/round-to-nearest-fp16 behavior.

GPSIMD is intentionally idle: it shares its SBUF port pair with DVE
(exclusive per-instruction lock), so any GPSIMD tensor work would
serialize against the DVE chain at worse per-element efficiency.

Startup is pipelined in two column halves (x DMA -> t -> sq/th ->
env -> Gh_1 per half) so the chain starts ~13us in. Output: fp16
scaled levels, level-major [128, NJ, E]; 4-level 2MB DMA groups (the
last group flushes as 2+2 levels around the correction). Host
multiplies by 2^{e_k}, upcasts to f32 and permutes to (batch, i, j).
"""

from contextlib import ExitStack

import numpy as np

import concourse.bacc as bacc
import concourse.mybir as mybir
import concourse.tile as tile
from concourse.bass_utils import run_bass_kernel_spmd

F32 = mybir.dt.float32
F16 = mybir.dt.float16
AF = mybir.ActivationFunctionType
ALU = mybir.AluOpType

NJ = 32          # number of Hermite orders
N_CORES = 8
B = 65536        # full batch
BC = B // N_CORES
E = BC * NJ // 128   # 2048 free elems per partition per core

KG = 4           # k-levels per DMA flush group

# Per-level scale exponents e_k: Gh_k = G_k / 2^{e_k}.  amp_k =
# max_{|s|<=5.1} |env*H_k| computed offline in float64, e_k = ceil(log2).
EXP = [0, 1, 2, 3, 4, 6, 8, 10, 12, 14, 16, 18, 20, 23, 25, 28,
       30, 33, 35, 37, 40, 43, 46, 48, 51, 54, 57, 59, 62, 65, 68, 71]
SIG = [float(2.0**e) for e in EXP]


def _build(e=E):
    nc = bacc.Bacc("TRN2", target_bir_lowering=False, debug=False)
    x_d = nc.dram_tensor("x", [128, e], F32, kind="ExternalInput").ap()
    om_d = nc.dram_tensor("om", [128, 1], F32, kind="ExternalInput").ap()
    out_d = nc.dram_tensor("out", [128, NJ * e], F16, kind="ExternalOutput").ap()

    n_groups = NJ // KG
    h = e // 2
    halves = [(0, h), (h, e)]
    with tile.TileContext(nc) as tc, ExitStack() as ctx:
        cpool = ctx.enter_context(tc.tile_pool(name="const", bufs=1))
        xp = ctx.enter_context(tc.tile_pool(name="xp", bufs=1))
        qp = ctx.enter_context(tc.tile_pool(name="qp", bufs=3))
        rp = ctx.enter_context(tc.tile_pool(name="rp", bufs=3))
        gp = ctx.enter_context(tc.tile_pool(name="gp", bufs=3))
        zp = ctx.enter_context(tc.tile_pool(name="zp", bufs=3))

        om2 = cpool.tile([128, 1], F32)
        nc.sync.dma_start(om2[:, :], om_d[:, :])
        nc.scalar.mul(om2[:, :], om2[:, :], 2.0)  # om2 = 2*omega

        # level-group tiles: [128, KG*e] fp16, level k in slice k%KG
        groups = [None] * n_groups

        def gh(k):
            return groups[k // KG][:, (k % KG) * e : (k % KG + 1) * e]

        def open_group(k):
            q, r = divmod(k, KG)
            if r == 0:
                groups[q] = gp.tile([128, KG * e], F16, name=f"grp{q}", tag="g")

        # ---- two-half pipelined startup ----
        x_t = xp.tile([128, e], F32)
        t_t = xp.tile([128, e], F32, name="t")
        th = xp.tile([128, e], F16, name="th")
        sq = xp.tile([128, e], F32, name="sq")
        open_group(0)
        open_group(1)
        for lo, hi in halves:
            nc.sync.dma_start(x_t[:, lo:hi], x_d[:, lo:hi])
            nc.scalar.mul(t_t[:, lo:hi], x_t[:, lo:hi], om2[:, 0:1])
            nc.vector.tensor_mul(sq[:, lo:hi], t_t[:, lo:hi], t_t[:, lo:hi])
            nc.scalar.activation(
                groups[0][:, lo:hi], sq[:, lo:hi], AF.Exp, scale=-0.125
            )  # Gh_0 = env = exp(-t^2/8)
            nc.scalar.copy(th[:, lo:hi], t_t[:, lo:hi])  # t_hat = fp16(t)
            # Gh_1 = (t/2)*env  (sigma_1 = 2)
            nc.vector.scalar_tensor_tensor(
                groups[0][:, e + lo : e + hi],
                t_t[:, lo:hi],
                0.5,
                groups[0][:, lo:hi],
                ALU.mult,
                ALU.mult,
            )

        # pre-scaled t_hat variants th/2^d (exact fp16 halvings) and the
        # t_lo correction inputs are deferred into early-chain DVE/ACT
        # bubble slots (first uses: ta2@k=5, ta3@k=13, tl/w1p/w1x/z1a@k>=30)
        ta = {}
        for dd in (1, 2, 3):
            ta[dd] = xp.tile([128, e], F16, name=f"ta{dd}")
        nc.vector.tensor_scalar_mul(ta[1][:, :], th[:, :], 0.5)
        tl = xp.tile([128, e], F16, name="tl")
        w1p = xp.tile([128, e], F16, name="w1p")
        w1x = xp.tile([128, e], F16, name="w1x")  # 1 - (t_hat/4)*t_lo
        z1a = xp.tile([128, e], F16, name="z1a")
        c1 = float((NJ - 1.0) * SIG[NJ - 2] / SIG[NJ - 1])

        def flush(k0, k1):  # DMA levels [k0, k1] (same group) to DRAM
            q = k0 // KG
            r0, r1 = k0 % KG, k1 % KG
            nc.sync.dma_start(
                out_d[:, k0 * e : (k1 + 1) * e],
                groups[q][:, r0 * e : (r1 + 1) * e],
            )

        z1 = None
        for k in range(2, NJ):
            open_group(k)
            dk = EXP[k] - EXP[k - 1]
            b = 2.0 * (k - 1) * SIG[k - 2] / SIG[k]
            q_t = qp.tile([128, e], F16, name=f"q{k}", tag="q")
            nc.vector.tensor_mul(q_t[:, :], ta[dk][:, :], gh(k - 1))
            r_t = rp.tile([128, e], F16, name=f"r{k}", tag="r")
            nc.scalar.mul(r_t[:, :], gh(k - 2), -b)
            if k == 2:  # DVE waits on r_2 here anyway — free slot
                nc.vector.scalar_tensor_tensor(
                    tl[:, :], th[:, :], -1.0, t_t[:, :], ALU.mult, ALU.add
                )
            if k == NJ - 1:
                break
            nc.vector.tensor_add(gh(k), q_t[:, :], r_t[:, :])
            if k == 2:
                nc.vector.tensor_mul(w1p[:, :], th[:, :], tl[:, :])
            elif k == 3:
                nc.vector.tensor_scalar_mul(ta[2][:, :], th[:, :], 0.25)
                nc.vector.tensor_scalar_mul(ta[3][:, :], th[:, :], 0.125)
            elif k == 20:  # ACT slack region; needed at k>=30
                nc.scalar.activation(
                    w1x[:, :], w1p[:, :], AF.Copy, bias=1.0, scale=-0.25
                )
                nc.scalar.mul(z1a[:, :], tl[:, :], c1)
            if k % KG == KG - 1 and k < NJ - 1:
                flush(k - 3, k)
            if k == NJ - 2:
                flush(k - 2, k - 1)  # 28, 29
                # z1 = (c1*t_lo)*Gh_30 — issue before the last level's ops
                z1 = zp.tile([128, e], F16, name="z1", tag="z")
                nc.vector.tensor_mul(z1[:, :], z1a[:, :], gh(k))
                flush(k, k)  # 30

        # final level with folded t_lo correction:
        #   u = q + r (uncorrected Gh_31); Gh_31 = u*(1 - (t_hat/4)*t_lo) + z1
        u_t = zp.tile([128, e], F16, name="u", tag="z")
        nc.vector.tensor_add(u_t[:, :], q_t[:, :], r_t[:, :])
        v_t = zp.tile([128, e], F16, name="v", tag="z")
        nc.vector.tensor_mul(v_t[:, :], u_t[:, :], w1x[:, :])
        nc.vector.tensor_add(gh(NJ - 1), v_t[:, :], z1[:, :])
        flush(NJ - 1, NJ - 1)

    nc.compile()
    return nc


_CACHED_NC = None


def kernel(x: np.ndarray, omega_kernel: np.ndarray, **run_kwargs) -> np.ndarray:
    global _CACHED_NC
    assert x.shape == (B, NJ, 1) and omega_kernel.shape == (1, 1), (
        x.shape,
        omega_kernel.shape,
    )
    x = np.ascontiguousarray(x, np.float32)
    om = np.ascontiguousarray(omega_kernel, np.float32)

    if _CACHED_NC is None:
        _CACHED_NC = _build()
    nc = _CACHED_NC

    in_maps = [
        {
            "x": x[c * BC : (c + 1) * BC].reshape(128, E),
            "om": np.ascontiguousarray(np.broadcast_to(om, (128, 1))),
        }
        for c in range(N_CORES)
    ]
    res = run_bass_kernel_spmd(nc, in_maps, core_ids=list(range(N_CORES)), **run_kwargs)
    sig = np.asarray(SIG, np.float32)  # [NJ]
    full = np.empty((B, NJ, NJ), np.float32)
    for c in range(N_CORES):
        arr = np.asarray(res.results[c]["out"]).reshape(128, NJ, E)
        out3 = arr.astype(np.float32) * sig[None, :, None]
        # scalar index = p*E + col ; out[b, i, j] = out3[p, j, col]
        full[c * BC : (c + 1) * BC] = (
            out3.transpose(0, 2, 1).reshape(BC, NJ, NJ)
        )
    if run_kwargs:
        return full, res
    return full
